# revision 2
# baseline (speedup 1.0000x reference)
"""Batched KNN (K=32) on 8 Trainium2 NeuronCores — chunked-candidate version.

Per core (= one contiguous batch block, since `batch` is sorted), over the
block's first 1024 rows x 1024 cols (the <=16 remaining rows/cols per block
are folded into the exact host-side merge):

  PE  : ps = x_blk @ x_blk.T - sq_j/2 in fp32 -- the -sq_j/2 comes from an
        augmented K=1 matmul (ones (x) -sq_j/2) in the same PSUM
        accumulation group, so no vector-engine work is spent on the bias.
  ACT : w = 2*ps per 512-col half (PSUM -> SBUF, frees the PSUM bank fast)
  DVE : per 256-col chunk: top-8 (max8 + max_index), match_replace,
        top-8 again -> exact top-16 (value, in-chunk index) per chunk,
        64 candidates per row.  Instructions round-robin across the 4
        chunks so dependent pairs are 4 slots apart.
  Host: appends exact candidates for the leftover columns, merges the
        per-row candidates into the top-32 by (d2, idx).  Rows where a
        chunk's deepest candidate reaches the merged top-32, and rows
        beyond the 8 device row-tiles, are recomputed exactly (~220 of
        8192 rows).

DVE is the saturated engine (PE finishes all tiles in about half the
kernel span), so the design minimizes DVE scans: ~4 full-width passes per
128-row tile vs 11 for the baseline 4-round max8/max_index/match_replace.
"""

import os
import sys

import numpy as np

for _p in ("/opt/trn_rl_repo", "/root/.axon_site/_ro/trn_rl_repo"):
    if os.path.isdir(_p) and _p not in sys.path:
        sys.path.append(_p)

K = 32
BIG = 1e30
N_CORES = 8
NCHUNK = 4
NC_PER_CHUNK = 16
T = 8          # device row-tiles per core
CW = 256       # candidate-chunk width
W = NCHUNK * CW

LAST_EXEC_NS = None

_NC_CACHE = {}


def _build_nc(D):
    import concourse.bass as bass  # noqa: F401
    from concourse import bacc, mybir
    from concourse.tile import TileContext

    f32 = mybir.dt.float32
    u16 = mybir.dt.uint16
    KC = D // 128
    assert D % 128 == 0

    P = T * 128
    nc = bacc.Bacc(None, target_bir_lowering=False)
    xt_d = nc.dram_tensor("xt", [D, W], f32, kind="ExternalInput")
    sqh_d = nc.dram_tensor("sqh", [1, W], f32, kind="ExternalInput")
    od_d = nc.dram_tensor("od", [P, 64], f32, kind="ExternalOutput")
    oi_d = nc.dram_tensor("oi", [P, 64], u16, kind="ExternalOutput")

    NMC = W // 512  # 512-wide matmul col-chunks

    with TileContext(nc) as tc:
        with tc.tile_pool(name="const", bufs=1) as cpool, \
             tc.tile_pool(name="work", bufs=4) as wpool, \
             tc.tile_pool(name="outp", bufs=4) as opool, \
             tc.tile_pool(name="psum", bufs=2, space="PSUM") as ppool:
            ones_sb = cpool.tile([1, 128], f32, tag="ones")
            nc.gpsimd.memset(ones_sb[:, :], 1.0)
            sqh_sb = cpool.tile([1, W], f32, tag="sqh")
            nc.sync.dma_start(sqh_sb[:, :], sqh_d[:, :])
            xt_sb = {}
            for mc in range(NMC):
                for k in range(KC):
                    xk = cpool.tile([128, 512], f32, tag=f"xt{k}_{mc}")
                    nc.sync.dma_start(
                        xk[:, :],
                        xt_d[k * 128:(k + 1) * 128,
                             mc * 512:(mc + 1) * 512])
                    xt_sb[(k, mc)] = xk

            for t in range(T):
                q0 = t * 128
                smc, sof = q0 // 512, q0 % 512
                ps = ppool.tile([128, W], f32, tag="ps")
                w = wpool.tile([128, W], f32, tag="w")
                # col-chunk-major with the K=1 bias matmul closing each
                # accumulation group, so the ACT copy (and then the DVE
                # chain) for cols [0,512) starts while cols [512,1024)
                # are still on the PE
                for mc in range(NMC):
                    for k in range(KC):
                        nc.tensor.matmul(
                            ps[:, mc * 512:(mc + 1) * 512],
                            xt_sb[(k, smc)][:, sof:sof + 128],
                            xt_sb[(k, mc)][:, :],
                            start=(k == 0),
                            stop=False,
                        )
                    nc.tensor.matmul(
                        ps[:, mc * 512:(mc + 1) * 512],
                        ones_sb[0:1, :],
                        sqh_sb[0:1, mc * 512:(mc + 1) * 512],
                        start=False,
                        stop=True,
                    )
                    nc.scalar.activation(
                        w[:, mc * 512:(mc + 1) * 512],
                        ps[:, mc * 512:(mc + 1) * 512],
                        mybir.ActivationFunctionType.Copy, scale=2.0)
                vals = opool.tile([128, 64], f32, tag="vals")
                inds = opool.tile([128, 64], u16, tag="inds")
                for c in range(NCHUNK):
                    cs = slice(c * CW, (c + 1) * CW)
                    vs = slice(c * 16, c * 16 + 8)
                    nc.vector.max(out=vals[:, vs], in_=w[:, cs])
                for c in range(NCHUNK):
                    cs = slice(c * CW, (c + 1) * CW)
                    vs = slice(c * 16, c * 16 + 8)
                    nc.vector.max_index(
                        out=inds[:, vs], in_max=vals[:, vs],
                        in_values=w[:, cs])
                for c in range(NCHUNK):
                    cs = slice(c * CW, (c + 1) * CW)
                    vs = slice(c * 16, c * 16 + 8)
                    nc.vector.match_replace(
                        out=w[:, cs], in_to_replace=vals[:, vs],
                        in_values=w[:, cs], imm_value=-BIG)
                for c in range(NCHUNK):
                    cs = slice(c * CW, (c + 1) * CW)
                    vs = slice(c * 16 + 8, c * 16 + 16)
                    nc.vector.max(out=vals[:, vs], in_=w[:, cs])
                for c in range(NCHUNK):
                    cs = slice(c * CW, (c + 1) * CW)
                    vs = slice(c * 16 + 8, c * 16 + 16)
                    nc.vector.max_index(
                        out=inds[:, vs], in_max=vals[:, vs],
                        in_values=w[:, cs])
                nc.sync.dma_start(od_d[q0:q0 + 128, :], vals[:, :])
                nc.sync.dma_start(oi_d[q0:q0 + 128, :], inds[:, :])
    nc.finalize()
    return nc


def kernel(x, batch):
    global LAST_EXEC_NS
    from concourse.bass_utils import run_bass_kernel_spmd

    x = np.ascontiguousarray(np.asarray(x), dtype=np.float32)
    b = np.asarray(batch)
    N, D = x.shape
    bounds = np.searchsorted(b, np.arange(N_CORES + 1))
    sizes = np.diff(bounds)
    assert int(sizes.max()) <= T * 128 + 128, "block larger than expected"
    ndev = T * 128  # rows/cols per core handled on device

    if D not in _NC_CACHE:
        _NC_CACHE[D] = _build_nc(D)
    nc = _NC_CACHE[D]

    sq = np.einsum("ij,ij->i", x, x, dtype=np.float32)
    in_maps = []
    for c in range(N_CORES):
        s, e = int(bounds[c]), int(bounds[c + 1])
        n = min(e - s, W)
        xt = np.zeros((D, W), np.float32)
        xt[:, :n] = x[s:s + n].T
        sqh = np.full((1, W), -BIG / 2, np.float32)
        sqh[0, :n] = -0.5 * sq[s:s + n]
        in_maps.append({"xt": xt, "sqh": sqh})

    trace = os.environ.get("KNN_TRACE", "0") == "1"
    res = run_bass_kernel_spmd(
        nc, in_maps, core_ids=list(range(N_CORES)), trace=trace)
    LAST_EXEC_NS = res.exec_time_ns

    # ---- host merge: device candidates + exact leftover-column cands ----
    NCAND = 64 + 16
    all_vals = np.full((N, NCAND), -np.float32(BIG), np.float32)
    all_gidx = np.zeros((N, NCAND), np.int64)
    rows = np.arange(N)
    blk = np.searchsorted(bounds, rows, side="right") - 1
    bstart = bounds[blk]
    bn = bounds[blk + 1] - bounds[blk]
    chunk = (np.arange(64) // NC_PER_CHUNK)[None, :]
    for c in range(N_CORES):
        s, e = int(bounds[c]), int(bounds[c + 1])
        n = min(e - s, ndev)
        if n > 0:
            all_vals[s:s + n, :64] = res.results[c]["od"][:n]
            lidx = res.results[c]["oi"][:n].astype(np.int64)
            all_gidx[s:s + n, :64] = s + lidx + chunk * CW
        # exact candidates for the block's columns beyond the device width
        nx = (e - s) - W
        if nx > 0:
            xcols = x[s + W:e]  # [nx, D]
            d2x = (sq[s:e, None] + sq[None, s + W:e]
                   - 2.0 * (x[s:e] @ xcols.T)).astype(np.float32)
            all_vals[s:e, 64:64 + nx] = sq[s:e, None] - d2x
            all_gidx[s:e, 64:64 + nx] = np.arange(s + W, e)[None, :]

    abscol = all_gidx - bstart[:, None]
    valid = (all_vals > -1e29) & (abscol < bn[:, None]) \
        & (all_gidx != rows[:, None])
    d2c = np.where(
        valid, (sq[:, None] - all_vals).astype(np.float32),
        np.float32(np.inf))
    # exact lexicographic (d2, idx) key; d2 is f32 so one f32-ulp gap
    # (>=6e-5 * 2^28 = 16k) dominates any idx < 8192
    key64 = d2c.astype(np.float64) * (1 << 28) + all_gidx
    order = np.argsort(key64, axis=1, kind="stable")
    ks = np.take_along_axis(key64, order, 1)
    dup_sorted = np.zeros((N, NCAND), bool)
    dup_sorted[:, 1:] = (ks[:, 1:] == ks[:, :-1]) & np.isfinite(ks[:, 1:])
    key2 = key64.copy()
    np.put_along_axis(key2, order, np.where(dup_sorted, np.inf, ks), 1)
    order2 = np.argsort(key2, axis=1, kind="stable")
    top = order2[:, :K]
    d2c2 = np.where(np.isfinite(key2), d2c, np.float32(np.inf))
    out_d = np.take_along_axis(d2c2, top, 1)
    out_i = np.take_along_axis(all_gidx, top, 1).astype(np.int32)
    nvalid = np.isfinite(key2).sum(1)

    # rows needing exact host recompute
    valid2 = np.isfinite(key2) & valid
    d2_32 = out_d[:, K - 1]
    suspect = (nvalid < K) | (rows - bstart >= ndev)
    for c in range(NCHUNK):
        sl = c * NC_PER_CHUNK + NC_PER_CHUNK - 1
        suspect |= valid2[:, sl] & (d2c[:, sl] <= d2_32)

    srows = np.nonzero(suspect)[0]
    for bb in range(N_CORES):
        s, e = int(bounds[bb]), int(bounds[bb + 1])
        rsel = srows[(srows >= s) & (srows < e)]
        if len(rsel) == 0:
            continue
        d2r = (sq[rsel, None] + sq[None, s:e]
               - 2.0 * (x[rsel] @ x[s:e].T)).astype(np.float32)
        d2r[np.arange(len(rsel)), rsel - s] = np.float32(BIG)
        o = np.argsort(d2r, axis=1, kind="stable")[:, :K]
        out_i[rsel] = (o + s).astype(np.int32)
        out_d[rsel] = np.take_along_axis(d2r, o, axis=1)

    return out_d, out_i.astype(np.int32)
